# revision 1
# baseline (speedup 1.0000x reference)
"""DPGN (gnn_message_passing) Trainium2 kernel — data-parallel over B on 8 cores.

Structure (see reference.py):
    pe  = PS(middle_node, point_edge)
    gen l=0..1:  pe = PS(point_node, pe);  dn = lrelu([pe[:,:, :S], dn] @ W_l^T + b_l)
    -> (dn_0, dn_1)

PS(v, ep): sim=(v_i-v_j)^2 ; h=lrelu(BN1(sim@w1)) ; h2=lrelu(BN2(h@w2)) ;
e=sigmoid(h2@w3+b3) ; epilogue(e, ep) (row normalisation).

Exploited structure:
  * e depends only on v: gen-1/2 share e(point_node) -> only two heavy cores.
  * e is SYMMETRIC: sim(i,j)=sim(j,i), so only j >= 16*floor(i/16) positions
    are computed: a per-batch "T tile" (all 10 diagonal 16x16 blocks, both
    orders, exact) + 9 shrinking "U pairs" (j >= block end, each unordered
    pair once).  BN2 batch stats stay exact by aggregating T packets once and
    U packets with weight 2.  The lower e-triangle is rebuilt by small PE
    transposes through HBM.
  * BN1 stats of sim@w1 have a closed form in per-node moments of v ->
    computed exactly on host (fp64).  BN2 stats on device (bn_stats) + one
    tiny [128x2] AllReduce per v across the 8 cores.
  * h (f16) kept in SBUF for point_node, spilled to HBM for middle_node;
    pass B recomputes h2 = w2 @ h on PE.
  * Phases are software-pipelined at work-item granularity: pass B(mid)
    interleaves into pass A(pt), mirror(mid) into pass B(pt), so the
    in-order per-engine queues always have independent work.

Device layout: channels on partitions; partitions 0:64 = rows 16p..16p+7,
64:128 = rows 16p+8..16p+15 (via a shifted copy of v^T).
"""

import numpy as np

import concourse.bass as bass
import concourse.bacc as bacc
import concourse.tile as tile
from concourse import mybir
from concourse.bass_utils import run_bass_kernel_spmd

F32 = mybir.dt.float32
F16 = mybir.dt.float16
AF = mybir.ActivationFunctionType
ALU = mybir.AluOpType
AX = mybir.AxisListType

B, N, C, S, G = 16, 160, 64, 80, 2
CH1 = 2 * C  # 128
BN_EPS = 1e-5
SLOPE = 0.01
N_CORES = 8
BL = B // N_CORES           # 2 local batches per core
NBLK = N // 16              # 10 row blocks
NTOT = B * N * N            # 409600

# --- symmetric tiling tables (per bl) ---
WU = [144 - 16 * p for p in range(9)]          # U-pair widths, p=0..8
OFF_U = []
_o = 1280                                       # T tile occupies [0,1280)
for _w in WU:
    OFF_U.append(_o)
    _o += 8 * _w
FLAT = _o                                       # 7040 cols per bl (per half)
assert FLAT == 7040

T_CHUNKS = [(0, 512, 4), (512, 512, 4), (1024, 256, 2)]  # (c0,cw,nblocks)


def _u_chunks(w):
    rp = min(8, 512 // w)
    out = []
    r = 0
    while r < 8:
        r1 = min(8, r + rp)
        out.append((r, r1))
        r = r1
    return out


U_CHUNKS = [_u_chunks(w) for w in WU]
U_SLOT = [0]
for _c in U_CHUNKS:
    U_SLOT.append(U_SLOT[-1] + len(_c))
N_TCH = len(T_CHUNKS)                       # 3 T chunks per bl
N_UCH = U_SLOT[-1]                          # 15 U chunks per bl
NT_POS = NBLK * 16 * 8                      # T positions per half per bl: 1280
NU_POS = 8 * sum(WU)                        # U positions per half per bl: 5760

# WORK item: (kind, p, pair_off, pair_sz, chunks[(c0,cw,extra)])
WORK = [("T", 0, 0, 1280, list(T_CHUNKS))]
for _p in range(9):
    _w = WU[_p]
    WORK.append(("U", _p, OFF_U[_p], 8 * _w,
                 [(r0 * _w, (r1 - r0) * _w, (r0, r1))
                  for (r0, r1) in U_CHUNKS[_p]]))

_PROG = None
TRACE = False
LAST_EXEC_NS = None
LAST_RESULTS = None


def _bn1_stats(v, w1):
    """Exact batch stats of einsum('bijc,oc->bijo', (v_i-v_j)^2, w1)."""
    Bv, Nv, _ = v.shape
    S1 = v.sum(1)
    S2 = (v ** 2).sum(1)
    P = np.einsum('bic,bid->bcd', v, v)
    Q = np.einsum('bic,bid->bcd', v ** 2, v)
    R = np.einsum('bic,bid->bcd', v ** 2, v ** 2)
    sim_sum = 2 * Nv * S2 - 2 * S1 ** 2
    M = (2 * Nv * R
         + 2 * np.einsum('bc,bd->bcd', S2, S2)
         + 4 * P ** 2
         - 4 * np.einsum('bcd,bd->bcd', Q, S1)
         - 4 * np.einsum('bdc,bc->bcd', Q, S1))
    n = Bv * Nv * Nv
    m1 = w1 @ (sim_sum.sum(0) / n)
    E2 = np.einsum('oc,cd,od->o', w1, M.sum(0) / n, w1)
    return m1, E2 - m1 ** 2


def build_program(n_cores=N_CORES, no_collective=False):
    nc = bacc.Bacc(None, target_bir_lowering=False, debug=False)

    def inp(name, shape, dt=F32):
        return nc.dram_tensor(name, list(shape), dt, kind="ExternalInput")

    VKS = ("mid", "pt")
    vshift = {vk: inp(f"vshift_{vk}", (128, BL, N)) for vk in VKS}
    vdup = {vk: inp(f"vdup_{vk}", (128, BL, N)) for vk in VKS}
    w1T = inp("w1T", (128, CH1), F16)
    w2T = inp("w2T", (CH1, C), F16)
    w3pair = inp("w3pair", (CH1, 2), F16)
    ab1 = {vk: inp(f"ab1_{vk}", (CH1, 2)) for vk in VKS}
    g2b2 = inp("g2b2", (C, 2))
    b3p = inp("b3p", (128, 1))
    point_edge = inp("point_edge", (BL, N, N))
    dnT0 = inp("dnT0", (S, BL, N))
    p2d_wa = inp("p2d_wa", (S, G, S))
    p2d_wb = inp("p2d_wb", (S, G, S))
    p2d_bias = inp("p2d_bias", (S, G))
    maskdiag = inp("maskdiag", (N, N))
    eyeplus = inp("eyeplus", (N, N))
    ident = inp("ident", (128, 128))

    out_dn = [nc.dram_tensor(f"out{l}", [BL, N, S], F32, kind="ExternalOutput")
              for l in range(G)]

    h_spill = nc.dram_tensor("h_spill", [128, BL, FLAT], F16)
    e_full = {vk: nc.dram_tensor(f"efull_{vk}", [BL, N, N], F32) for vk in VKS}
    cc_in = {vk: nc.dram_tensor(f"ccin_{vk}", [128, 2], F32) for vk in VKS}
    cc_out = {vk: nc.dram_tensor(f"ccout_{vk}", [128, 2], F32,
                                 addr_space="Shared") for vk in VKS}
    groups = [list(range(n_cores))]

    with tile.TileContext(nc) as tc, \
         tc.tile_pool(name="singles", bufs=1) as singles, \
         tc.tile_pool(name="hpt", bufs=1) as hptpool:

        dma = nc.default_dma_engine

        def load(t, shape, dt=F32, tag=None):
            sb = singles.tile(list(shape), dt, tag=tag or t.name,
                              name=tag or t.name)
            dma.dma_start(out=sb, in_=t[tuple(slice(0, s) for s in shape)])
            return sb

        vshift_sb = {vk: load(vshift[vk], (128, BL, N)) for vk in VKS}
        vdup_sb = {vk: load(vdup[vk], (128, BL, N)) for vk in VKS}
        w1T_sb = load(w1T, (128, CH1), F16)
        w2T_sb = load(w2T, (CH1, C), F16)
        w3p_sb = load(w3pair, (CH1, 2), F16)
        ab1_sb = {vk: load(ab1[vk], (CH1, 2)) for vk in VKS}
        g2b2_sb = load(g2b2, (C, 2))
        b3_sb = load(b3p, (128, 1))
        dnT0_sb = load(dnT0, (S, BL, N))
        p2dwa_sb = load(p2d_wa, (S, G, S))
        p2dwb_sb = load(p2d_wb, (S, G, S))
        p2db_sb = load(p2d_bias, (S, G))
        ident_sb = load(ident, (128, 128))
        mask_sb = [load(maskdiag, (128, N), tag="mask0"),
                   singles.tile([32, N], F32, tag="mask1", name="mask1")]
        dma.dma_start(out=mask_sb[1], in_=maskdiag[128:160, :])
        eyep_sb = [load(eyeplus, (128, N), tag="eyep0"),
                   singles.tile([32, N], F32, tag="eyep1", name="eyep1")]
        dma.dma_start(out=eyep_sb[1], in_=eyeplus[128:160, :])

        hpt_all = hptpool.tile([128, BL, FLAT], F16, tag="hpt_all")
        stats_T = {vk: singles.tile([128, N_TCH * BL, 6], F32,
                                    tag=f"statsT_{vk}",
                                    name=f"statsT_{vk}") for vk in VKS}
        stats_U = {vk: singles.tile([128, N_UCH * BL, 6], F32,
                                    tag=f"statsU_{vk}",
                                    name=f"statsU_{vk}") for vk in VKS}

        with tc.tile_pool(name="wpa", bufs=3) as wpa, \
             tc.tile_pool(name="wpb", bufs=3) as wpb, \
             tc.tile_pool(name="pcp", bufs=2) as pcp, \
             tc.tile_pool(name="pck", bufs=1) as pck, \
             tc.tile_pool(name="pph1", bufs=2, space="PSUM") as pph1, \
             tc.tile_pool(name="pph2", bufs=2, space="PSUM") as pph2:

            # ---------------- pass A item ----------------
            def emit_a(vk, bl, widx):
                resident = (vk == "pt")
                kind, p, poff, psz, chunks = WORK[widx]
                simtmp = wpa.tile([128, 1280], F16, tag="simtmp", bufs=6)
                sim = wpa.tile([128, 1280], F16, tag="sim", bufs=6)
                if kind == "T":
                    in0 = (vshift_sb[vk][:, bl, :]
                           .rearrange("c (p i) -> c p i", i=16)
                           [:, :, 0:8].unsqueeze(-1)
                           .broadcast_to([128, NBLK, 8, 16]))
                    in1 = (vdup_sb[vk][:, bl, :]
                           .rearrange("c (p w) -> c p w", w=16)
                           .unsqueeze(2)
                           .broadcast_to([128, NBLK, 8, 16]))
                    st = simtmp[:, :1280].rearrange(
                        "c (p i w) -> c p i w", i=8, w=16)
                    sv = sim[:, :1280].rearrange(
                        "c (p i w) -> c p i w", i=8, w=16)
                else:
                    w = WU[p]
                    in0 = (vshift_sb[vk][:, bl, 16 * p:16 * p + 8]
                           .unsqueeze(-1).broadcast_to([128, 8, w]))
                    in1 = (vdup_sb[vk][:, bl, 16 * p + 16:N]
                           .unsqueeze(1).broadcast_to([128, 8, w]))
                    st = simtmp[:, :8 * w].rearrange("c (i w) -> c i w", w=w)
                    sv = sim[:, :8 * w].rearrange("c (i w) -> c i w", w=w)
                sub_eng = nc.gpsimd if bl == 0 else nc.vector
                sub_eng.tensor_sub(st, in0, in1)
                nc.vector.tensor_mul(sv, st, st)

                hAB = wpa.tile([128, 2, 1280], F16, tag="hAB")
                hA = hAB[:, 0, :psz]
                hB = hAB[:, 1, :psz]
                for half, hdst in ((0, hA), (1, hB)):
                    rows = sim[64 * half:64 * half + 64, :psz]
                    for (c0, cw, _x) in chunks:
                        h1 = pph1.tile([128, 512], F32, tag="h1")
                        nc.tensor.matmul(
                            h1[:, :cw],
                            lhsT=w1T_sb[64 * half:64 * half + 64, :],
                            rhs=rows[:, c0:c0 + cw],
                            start=True, stop=True)
                        nc.scalar.activation(
                            out=hdst[:, c0:c0 + cw], in_=h1[:, :cw],
                            func=AF.Prelu,
                            bias=ab1_sb[vk][:, 1:2],
                            scale=ab1_sb[vk][:, 0:1],
                            alpha=SLOPE)

                if resident:
                    h2d = hpt_all[:, bl, poff:poff + psz]
                else:
                    h2AB = wpa.tile([128, 1280], F16, tag="h2AB")
                    h2d = h2AB[:, :psz]
                for k, (c0, cw, _x) in enumerate(chunks):
                    h2 = pph2.tile([128, 512], F32, tag="h2")
                    nc.tensor.matmul(h2[0:64, :cw], lhsT=w2T_sb,
                                     rhs=hA[:, c0:c0 + cw],
                                     start=True, stop=True)
                    nc.tensor.matmul(h2[64:128, :cw], lhsT=w2T_sb,
                                     rhs=hB[:, c0:c0 + cw],
                                     start=True, stop=True)
                    if k % 2 == 0:
                        nc.scalar.copy(h2d[:, c0:c0 + cw], h2[:, :cw])
                    else:
                        nc.vector.tensor_copy(h2d[:, c0:c0 + cw],
                                              h2[:, :cw])
                    if kind == "T":
                        dst = stats_T[vk][:, N_TCH * bl + k, :]
                    else:
                        dst = stats_U[vk][:, N_UCH * bl + U_SLOT[p] + k, :]
                    nc.vector.bn_stats(out=dst, in_=h2d[:, c0:c0 + cw])
                if not resident:
                    dma.dma_start(out=h_spill[:, bl, poff:poff + psz],
                                  in_=h2AB[:, :psz])

            # ------------- stats reduce / collective -------------
            def reduce_sums(vk):
                nT = float(NT_POS * BL)
                nU2 = float(2 * NU_POS * BL)
                with tc.tile_pool(name=f"st_{vk}", bufs=1) as sp:
                    mvT = sp.tile([128, 2], F32, tag="mvT")
                    nc.vector.bn_aggr(out=mvT, in_=stats_T[vk])
                    mvU = sp.tile([128, 2], F32, tag="mvU")
                    nc.vector.bn_aggr(out=mvU, in_=stats_U[vk])
                    sums = sp.tile([128, 2], F32, tag="sums")
                    tmpU = sp.tile([128, 1], F32, tag="tmpU")
                    nc.vector.tensor_scalar_mul(tmpU, mvU[:, 0:1], nU2)
                    nc.vector.tensor_scalar_mul(sums[:, 0:1], mvT[:, 0:1], nT)
                    nc.vector.tensor_add(sums[:, 0:1], sums[:, 0:1], tmpU)
                    msqT = sp.tile([128, 1], F32, tag="msqT")
                    nc.vector.tensor_mul(msqT, mvT[:, 0:1], mvT[:, 0:1])
                    nc.vector.tensor_add(msqT, msqT, mvT[:, 1:2])
                    msqU = sp.tile([128, 1], F32, tag="msqU")
                    nc.vector.tensor_mul(msqU, mvU[:, 0:1], mvU[:, 0:1])
                    nc.vector.tensor_add(msqU, msqU, mvU[:, 1:2])
                    nc.vector.tensor_scalar_mul(msqT, msqT, nT)
                    nc.vector.tensor_scalar_mul(msqU, msqU, nU2)
                    nc.vector.tensor_add(sums[:, 1:2], msqT, msqU)
                    dma.dma_start(out=cc_in[vk][:, :], in_=sums)

            def collective(vk):
                if no_collective:
                    dma.dma_start(out=cc_out[vk][:, :], in_=cc_in[vk][:, :])
                else:
                    nc.gpsimd.collective_compute(
                        "AllReduce", ALU.add, replica_groups=groups,
                        ins=[cc_in[vk][:, :]], outs=[cc_out[vk][:, :]])

            # ------------- alpha2 / beta2 (gpsimd only: no queue stalls) ----
            def compute_ab2(vk):
                I32 = mybir.dt.int32
                with tc.tile_pool(name=f"ab2w_{vk}", bufs=1) as sp:
                    gs = sp.tile([128, 2], F32, tag="gs")
                    dma.dma_start(out=gs, in_=cc_out[vk][:, :])
                    bot = sp.tile([C, 2], F32, tag="bot")
                    dma.dma_start(out=bot, in_=gs[64:128, :])
                    tot = sp.tile([C, 2], F32, tag="tot")
                    nc.gpsimd.tensor_add(tot, gs[0:64, :], bot)
                    mE = sp.tile([C, 2], F32, tag="mE")
                    nc.gpsimd.tensor_scalar_mul(mE, tot, 1.0 / NTOT)
                    xe = sp.tile([C, 1], F32, tag="xe")
                    nc.gpsimd.tensor_mul(xe, mE[:, 0:1], mE[:, 0:1])
                    nc.gpsimd.tensor_sub(xe, mE[:, 1:2], xe)
                    nc.gpsimd.tensor_scalar_add(xe, xe, BN_EPS)
                    # rsqrt = exp(-0.5*ln(x)) + two Newton polishes on gpsimd
                    y = sp.tile([C, 1], F32, tag="y")
                    lx = sp.tile([C, 1], F32, tag="lx")
                    eps_t = sp.tile([C, 1], F32, tag="eps_t")
                    nc.gpsimd.memset(eps_t, 0.0)
                    nc.scalar.activation(out=lx, in_=xe, func=AF.Ln,
                                         bias=eps_t)
                    nc.gpsimd.tensor_scalar_mul(lx, lx, -0.5)
                    nc.scalar.activation(out=y, in_=lx, func=AF.Exp,
                                         bias=eps_t)
                    t1 = sp.tile([C, 1], F32, tag="t1")
                    for _ in range(2):
                        nc.gpsimd.tensor_mul(t1, y, y)
                        nc.gpsimd.tensor_mul(t1, t1, xe)
                        nc.gpsimd.tensor_scalar(t1, t1, -0.5, 1.5,
                                                ALU.mult, ALU.add)
                        nc.gpsimd.tensor_mul(y, y, t1)
                    ab2 = sp.tile([C, 2], F32, tag="ab2")
                    nc.gpsimd.tensor_mul(ab2[:, 0:1], y, g2b2_sb[:, 0:1])
                    t2 = sp.tile([C, 1], F32, tag="t2")
                    nc.gpsimd.tensor_mul(t2, mE[:, 0:1], ab2[:, 0:1])
                    nc.gpsimd.tensor_sub(ab2[:, 1:2], g2b2_sb[:, 1:2], t2)
                    ab2p = singles.tile([128, 2], F32, tag=f"ab2p_{vk}",
                                        name=f"ab2p_{vk}")
                    dma.dma_start(out=ab2p[0:64, :], in_=ab2)
                    dma.dma_start(out=ab2p[64:128, :], in_=ab2)
                return ab2p

            # ---------------- pass B item ----------------
            def emit_b(vk, ab2p, bl, widx):
                resident = (vk == "pt")
                kind, p, poff, psz, chunks = WORK[widx]
                if resident:
                    h2s = hpt_all[:, bl, poff:poff + psz]
                else:
                    h2AB = wpb.tile([128, 1280], F16, tag="hABr", bufs=5)
                    dma.dma_start(out=h2AB[:, :psz],
                                  in_=h_spill[:, bl, poff:poff + psz])
                    h2s = h2AB[:, :psz]
                for ci, (c0, cw, extra) in enumerate(chunks):
                    act_on_vec = (not resident) or ci % 2 == 1
                    hh = wpb.tile([128, 512], F16, tag="hh")
                    if not act_on_vec:
                        nc.scalar.activation(
                            out=hh[:, :cw], in_=h2s[:, c0:c0 + cw],
                            func=AF.Prelu,
                            bias=ab2p[:, 1:2], scale=ab2p[:, 0:1],
                            alpha=SLOPE)
                    else:
                        hy = wpb.tile([128, 512], F32, tag="hy")
                        nc.vector.tensor_scalar(hy[:, :cw],
                                                h2s[:, c0:c0 + cw],
                                                ab2p[:, 0:1], ab2p[:, 1:2],
                                                ALU.mult, ALU.add)
                        nc.vector.scalar_tensor_tensor(
                            out=hh[:, :cw], in0=hy[:, :cw], scalar=SLOPE,
                            in1=hy[:, :cw], op0=ALU.mult, op1=ALU.max)
                    e_pre = pph2.tile([2, 512], F32, tag="epre")
                    nc.tensor.matmul(e_pre[:, :cw], lhsT=w3p_sb,
                                     rhs=hh[:, :cw], start=True, stop=True)
                    e_sb = wpb.tile([2, 512], F32, tag="esb")
                    if act_on_vec:
                        nc.scalar.copy(e_sb[:, :cw], e_pre[:, :cw])
                    else:
                        nc.vector.tensor_copy(e_sb[:, :cw], e_pre[:, :cw])
                    if kind == "T":
                        for q in range(extra):
                            blk = c0 // 128 + q
                            dst = (e_full[vk]
                                   [bl, 16 * blk:16 * blk + 16,
                                    16 * blk:16 * blk + 16]
                                   .rearrange("(h i) w -> h i w", h=2))
                            src = (e_sb[:, 128 * q:128 * q + 128]
                                   .rearrange("h (i w) -> h i w", w=16))
                            dma.dma_start(out=dst, in_=src)
                    else:
                        w = WU[p]
                        r0, r1 = extra
                        dst = (e_full[vk]
                               [bl, 16 * p:16 * p + 16, 16 * p + 16:N]
                               .rearrange("(h i) w -> h i w", h=2)
                               [:, r0:r1, :])
                        src = e_sb[:, :cw].rearrange("h (i w) -> h i w", w=w)
                        dma.dma_start(out=dst, in_=src)

            # ------------- mirror item (lower triangle of e) -------------
            def emit_m(vk, bl, p):
                w = WU[p]
                strip = wpb.tile([16, 144], F32, tag="strip")
                dma.dma_start(out=strip[:, :w],
                              in_=e_full[vk][bl, 16 * p:16 * p + 16,
                                             16 * p + 16:N])
                woff = 0
                while woff < w:
                    wc = min(128, w - woff)
                    ps = pph2.tile([128, 512], F32, tag="h2r")
                    nc.tensor.matmul(ps[:wc, :16],
                                     lhsT=strip[:, woff:woff + wc],
                                     rhs=ident_sb[0:16, 0:16],
                                     is_transpose=True, start=True, stop=True)
                    so = wpb.tile([128, 16], F32, tag="strip_out")
                    nc.scalar.copy(so[:wc, :], ps[:wc, :16])
                    dma.dma_start(
                        out=e_full[vk][bl, 16 * p + 16 + woff:
                                       16 * p + 16 + woff + wc,
                                       16 * p:16 * p + 16],
                        in_=so[:wc, :])
                    woff += wc

            # ---------------- schedule ----------------
            NW = len(WORK)
            A_items = [(bl, w) for bl in range(BL) for w in range(NW)]

            for (bl, w) in A_items:
                emit_a("mid", bl, w)
            reduce_sums("mid")
            for (bl, w) in A_items[:NW]:
                emit_a("pt", bl, w)
            collective("mid")
            emit_a("pt", *A_items[NW])
            ab2p_mid = compute_ab2("mid")
            rest = A_items[NW + 1:]
            bi = 0
            for j, (bl, w) in enumerate(rest):
                emit_a("pt", bl, w)
                while bi < len(A_items) * (j + 1) // len(rest):
                    emit_b("mid", ab2p_mid, *A_items[bi])
                    bi += 1
            while bi < len(A_items):
                emit_b("mid", ab2p_mid, *A_items[bi])
                bi += 1
            reduce_sums("pt")
            collective("pt")
            for bl in range(BL):
                for p in range(9):
                    emit_m("mid", bl, p)

            # ---------------- phase C (staged) ----------------
            def epilogue(e_tiles, ep_tiles, tag):
                pe_tiles = []
                for blk, pdim in ((0, 128), (1, 32)):
                    e_t, ep_t = e_tiles[blk], ep_tiles[blk]
                    epm = pcp.tile([pdim, N], F32, tag=f"epm{blk}")
                    rs = pcp.tile([pdim, 1], F32, tag=f"rs{blk}")
                    nc.vector.scalar_tensor_tensor(
                        out=epm, in0=ep_t, scalar=1.0,
                        in1=mask_sb[blk][:pdim, :],
                        op0=ALU.mult, op1=ALU.mult, accum_out=rs)
                    x = pcp.tile([pdim, N], F32, tag=f"x{blk}_{tag}")
                    xs = pcp.tile([pdim, 1], F32, tag=f"xs{blk}")
                    nc.vector.scalar_tensor_tensor(
                        out=x, in0=e_t, scalar=1.0, in1=epm,
                        op0=ALU.mult, op1=ALU.mult, accum_out=xs)
                    nc.vector.tensor_scalar_max(xs, xs, 1e-12)
                    rxs = pcp.tile([pdim, 1], F32, tag=f"rxs{blk}")
                    nc.vector.reciprocal(rxs, xs)
                    nc.vector.tensor_mul(rxs, rxs, rs)
                    x2 = pcp.tile([pdim, N], F32, tag=f"x2{blk}_{tag}")
                    rs2 = pcp.tile([pdim, 1], F32, tag=f"rs2{blk}")
                    nc.vector.scalar_tensor_tensor(
                        out=x2, in0=x, scalar=rxs,
                        in1=eyep_sb[blk][:pdim, :],
                        op0=ALU.mult, op1=ALU.add, accum_out=rs2)
                    rrs2 = pcp.tile([pdim, 1], F32, tag=f"rrs2{blk}")
                    nc.vector.reciprocal(rrs2, rs2)
                    nc.vector.tensor_scalar_mul(x2, x2, rrs2)
                    pe_tiles.append(x2)
                return pe_tiles

            def pe_transpose(src_ap, pdim, fdim):
                ps = pph2.tile([128, 512], F32, tag="h2r")
                nc.tensor.matmul(ps[:fdim, :pdim], lhsT=src_ap,
                                 rhs=ident_sb[:pdim, :pdim],
                                 is_transpose=True, start=True, stop=True)
                dst = pcp.tile([fdim, pdim], F32, tag=f"tps{fdim}_{pdim}")
                nc.vector.tensor_copy(dst, ps[:fdim, :pdim])
                return dst

            CST = {}

            def load_blocks(bl, src, tagp, sigmoid=False):
                t0 = pck.tile([128, N], F32, tag=f"{tagp}0_{bl}")
                dma.dma_start(out=t0, in_=src[0:128, :])
                t1 = pck.tile([32, N], F32, tag=f"{tagp}1_{bl}")
                dma.dma_start(out=t1, in_=src[128:160, :])
                if sigmoid:
                    nc.scalar.activation(out=t0, in_=t0, func=AF.Sigmoid,
                                         bias=b3_sb[0:128])
                    nc.scalar.activation(out=t1, in_=t1, func=AF.Sigmoid,
                                         bias=b3_sb[0:32])
                return [t0, t1]

            def c_load_mid(bl):
                CST[(bl, "emid")] = load_blocks(
                    bl, e_full["mid"][bl], "emid", sigmoid=True)
                CST[(bl, "ep0")] = load_blocks(bl, point_edge[bl], "ep0")

            def c_load_pt(bl):
                CST[(bl, "ept")] = load_blocks(
                    bl, e_full["pt"][bl], "ept", sigmoid=True)

            def c_pe1(bl):
                CST[(bl, "pe1")] = epilogue(CST[(bl, "emid")],
                                            CST[(bl, "ep0")], f"pe1_{bl}")

            def c_pe2(bl):
                CST[(bl, "pe2")] = epilogue(CST[(bl, "ept")],
                                            CST[(bl, "pe1")], f"pe2_{bl}")

            def c_pe3(bl):
                CST[(bl, "pe3")] = epilogue(CST[(bl, "ept")],
                                            CST[(bl, "pe2")], f"pe3_{bl}")

            def c_p2d(bl, l):
                pe_t = CST[(bl, "pe2" if l == 0 else "pe3")]
                xT = pck.tile([S, N], F32, tag=f"xT_{bl}_{l}")
                t0 = pe_transpose(pe_t[0][:, 0:S], 128, S)
                nc.vector.tensor_copy(xT[:, 0:128], t0)
                t1 = pe_transpose(pe_t[1][:, 0:S], 32, S)
                nc.vector.tensor_copy(xT[:, 128:160], t1)
                dnT = dnT0_sb[:, bl, :] if l == 0 else CST[(bl, "dn")]
                mm = pph2.tile([128, 512], F32, tag="h2r")
                nc.tensor.matmul(mm[:S, :N], lhsT=p2dwa_sb[:, l, :], rhs=xT,
                                 start=True, stop=False)
                nc.tensor.matmul(mm[:S, :N], lhsT=p2dwb_sb[:, l, :], rhs=dnT,
                                 start=False, stop=True)
                dn_new = pck.tile([S, N], F32, tag=f"dnT_{bl}_{l}")
                dn_y = pck.tile([S, N], F32, tag=f"dny_{bl}_{l}")
                nc.vector.tensor_scalar_add(dn_y, mm[:S, :N],
                                            p2db_sb[:, l:l + 1])
                nc.vector.scalar_tensor_tensor(
                    out=dn_new, in0=dn_y, scalar=SLOPE, in1=dn_y,
                    op0=ALU.mult, op1=ALU.max)
                CST[(bl, "dn")] = dn_new
                o0 = pe_transpose(dn_new[:, 0:128], S, 128)
                dma.dma_start(out=out_dn[l][bl, 0:128, :], in_=o0)
                o1 = pe_transpose(dn_new[:, 128:160], S, 32)
                dma.dma_start(out=out_dn[l][bl, 128:160, :], in_=o1)

            # fill the AllReduce(pt) latency with the mid-edge epilogues
            c_load_mid(0)
            c_pe1(0)
            c_load_mid(1)
            c_pe1(1)
            ab2p_pt = compute_ab2("pt")

            C_STAGES = [c_load_pt, c_pe2, c_pe3,
                        lambda bl: c_p2d(bl, 0), lambda bl: c_p2d(bl, 1)]

            def run_c_stage(bl, idx):
                if idx % 2 == 0 and idx // 2 < len(C_STAGES):
                    C_STAGES[idx // 2](bl)

            for w in range(NW):
                emit_b("pt", ab2p_pt, 0, w)
            for p in range(9):
                emit_m("pt", 0, p)
            for j, w in enumerate(range(NW)):
                emit_b("pt", ab2p_pt, 1, w)
                run_c_stage(0, j)
            for p in range(9):
                emit_m("pt", 1, p)
            for idx in range(10):
                run_c_stage(1, idx)

    nc.compile()
    return nc


def _prep_maps(middle_node, point_node, distribution_node, distribution_edge,
               point_edge, w1, g1, b1, w2, g2, b2, w3, b3, p2d_w, p2d_b,
               n_cores=N_CORES):
    f4 = np.float32
    middle_node = np.asarray(middle_node)
    point_node = np.asarray(point_node)

    def vt_pair(v_local):
        vT = np.transpose(v_local, (0, 2, 1)).astype(f4)      # [BL, C, N]
        sh = np.concatenate([vT[:, :, 8:], np.zeros((BL, C, 8), f4)], axis=2)
        vshift = np.concatenate([vT, sh], axis=1)             # [BL, 128, N]
        vdup = np.concatenate([vT, vT], axis=1)
        # -> [128, BL, N]
        return (np.ascontiguousarray(np.transpose(vshift, (1, 0, 2))),
                np.ascontiguousarray(np.transpose(vdup, (1, 0, 2))))

    def ab1_for(v):
        m1, var1 = _bn1_stats(v.astype(np.float64), np.asarray(w1, np.float64))
        a = np.asarray(g1, np.float64) / np.sqrt(var1 + BN_EPS)
        bb = np.asarray(b1, np.float64) - m1 * a
        return np.ascontiguousarray(np.stack([a, bb], axis=1).astype(f4))

    ab1_mid = ab1_for(middle_node)
    ab1_pt = ab1_for(point_node)

    w1T_h = np.ascontiguousarray(np.concatenate(
        [np.asarray(w1).T, np.asarray(w1).T], axis=0).astype(np.float16))
    w2T_h = np.ascontiguousarray(np.asarray(w2).T.astype(np.float16))
    w3pair_h = np.zeros((CH1, 2), np.float16)
    w3pair_h[0:C, 0] = np.asarray(w3).astype(np.float16)
    w3pair_h[C:CH1, 1] = np.asarray(w3).astype(np.float16)
    g2b2_h = np.ascontiguousarray(np.stack([np.asarray(g2), np.asarray(b2)],
                                           axis=1).astype(f4))
    b3p_h = np.full((128, 1), float(np.asarray(b3)), f4)
    pw = np.asarray(p2d_w)
    p2d_wa_h = np.ascontiguousarray(
        np.transpose(pw[:, :, 0:S], (2, 0, 1)).astype(f4))      # [S,G,S]
    p2d_wb_h = np.ascontiguousarray(
        np.transpose(pw[:, :, S:2 * S], (2, 0, 1)).astype(f4))  # [S,G,S]
    p2d_bias_h = np.ascontiguousarray(np.asarray(p2d_b).T.astype(f4))
    maskdiag_h = (1.0 - np.eye(N)).astype(f4)
    eyeplus_h = (np.eye(N) + 1e-6).astype(f4)
    ident_h = np.eye(128, dtype=f4)

    maps = []
    for c in range(n_cores):
        sl = slice(c * BL, (c + 1) * BL)
        vs_m, vd_m = vt_pair(middle_node[sl])
        vs_p, vd_p = vt_pair(point_node[sl])
        dnT0_h = np.ascontiguousarray(
            np.transpose(np.asarray(distribution_node)[sl], (2, 0, 1))
            .astype(f4))                                        # [S,BL,N]
        maps.append(dict(
            vshift_mid=vs_m, vdup_mid=vd_m, vshift_pt=vs_p, vdup_pt=vd_p,
            w1T=w1T_h, w2T=w2T_h, w3pair=w3pair_h,
            ab1_mid=ab1_mid, ab1_pt=ab1_pt, g2b2=g2b2_h, b3p=b3p_h,
            point_edge=np.ascontiguousarray(
                np.asarray(point_edge)[sl].astype(f4)),
            dnT0=dnT0_h, p2d_wa=p2d_wa_h, p2d_wb=p2d_wb_h,
            p2d_bias=p2d_bias_h,
            maskdiag=maskdiag_h, eyeplus=eyeplus_h, ident=ident_h,
        ))
    return maps


def kernel(**inputs):
    global _PROG, LAST_EXEC_NS, LAST_RESULTS
    if _PROG is None:
        _PROG = build_program()
    maps = _prep_maps(**inputs)
    res = run_bass_kernel_spmd(_PROG, maps, core_ids=list(range(N_CORES)),
                               trace=TRACE)
    LAST_EXEC_NS = res.exec_time_ns
    LAST_RESULTS = res
    outs = []
    for l in range(G):
        outs.append(np.concatenate([res.results[c][f"out{l}"]
                                    for c in range(N_CORES)], axis=0))
    return tuple(outs)



# revision 14
# speedup vs baseline: 1.1019x; 1.1019x over previous
"""DPGN (gnn_message_passing) Trainium2 kernel — data-parallel over B on 8 cores.

Structure (see reference.py):
    pe  = PS(middle_node, point_edge)
    gen l=0..1:  pe = PS(point_node, pe);  dn = lrelu([pe[:,:, :S], dn] @ W_l^T + b_l)
    -> (dn_0, dn_1)

PS(v, ep): sim=(v_i-v_j)^2 ; h=lrelu(BN1(sim@w1)) ; h2=lrelu(BN2(h@w2)) ;
e=sigmoid(h2@w3+b3) ; epilogue(e, ep) (row normalisation).

Exploited structure:
  * e depends only on v: gen-1/2 share e(point_node) -> only two heavy cores.
  * e is SYMMETRIC: sim(i,j)=sim(j,i), so only j >= 16*floor(i/16) positions
    are computed: a per-batch "T tile" (all 10 diagonal 16x16 blocks, both
    orders, exact) + 9 shrinking "U pairs" (j >= block end, each unordered
    pair once).  BN2 batch stats stay exact by aggregating T packets once and
    U packets with weight 2.  The lower e-triangle is rebuilt by small PE
    transposes through HBM.
  * BN1 stats of sim@w1 have a closed form in per-node moments of v ->
    computed exactly on host (fp64).  BN2 stats on device (bn_stats) + one
    tiny [128x2] AllReduce per v across the 8 cores.
  * h (f16) kept in SBUF for point_node, spilled to HBM for middle_node;
    pass B recomputes h2 = w2 @ h on PE.
  * Phases are software-pipelined at work-item granularity: pass B(mid)
    interleaves into pass A(pt), mirror(mid) into pass B(pt), so the
    in-order per-engine queues always have independent work.

Device layout: channels on partitions; partitions 0:64 = rows 16p..16p+7,
64:128 = rows 16p+8..16p+15 (via a shifted copy of v^T).
"""

import numpy as np

import concourse.bass as bass
import concourse.bacc as bacc
import concourse.tile as tile
from concourse import mybir
from concourse.bass_utils import run_bass_kernel_spmd

F32 = mybir.dt.float32
F16 = mybir.dt.float16
AF = mybir.ActivationFunctionType
ALU = mybir.AluOpType
AX = mybir.AxisListType

B, N, C, S, G = 16, 160, 64, 80, 2
CH1 = 2 * C  # 128
BN_EPS = 1e-5
SLOPE = 0.01
N_CORES = 8
BL = B // N_CORES           # 2 local batches per core
NBLK = N // 16              # 10 row blocks
NTOT = B * N * N            # 409600

# --- symmetric tiling tables (per bl) ---
WU = [144 - 16 * p for p in range(9)]          # U-pair widths, p=0..8
OFF_U = []
_o = 1280                                       # T tile occupies [0,1280)
for _w in WU:
    OFF_U.append(_o)
    _o += 8 * _w
FLAT = _o                                       # 7040 cols per bl (per half)
assert FLAT == 7040

T_CHUNKS = [(0, 512, 4), (512, 512, 4), (1024, 256, 2)]  # (c0,cw,nblocks)


def _u_chunks(w):
    rp = min(8, 512 // w)
    out = []
    r = 0
    while r < 8:
        r1 = min(8, r + rp)
        out.append((r, r1))
        r = r1
    return out


U_CHUNKS = [_u_chunks(w) for w in WU]
U_SLOT = [0]
for _c in U_CHUNKS:
    U_SLOT.append(U_SLOT[-1] + len(_c))
N_TCH = len(T_CHUNKS)                       # 3 T chunks per bl
N_UCH = U_SLOT[-1]                          # 15 U chunks per bl
NT_POS = NBLK * 16 * 8                      # T positions per half per bl: 1280
NU_POS = 8 * sum(WU)                        # U positions per half per bl: 5760

# WORK item: (kind, p, pair_off, pair_sz, chunks[(c0,cw,extra)])
WORK = [("T", 0, 0, 1280, list(T_CHUNKS))]
for _p in range(9):
    _w = WU[_p]
    WORK.append(("U", _p, OFF_U[_p], 8 * _w,
                 [(r0 * _w, (r1 - r0) * _w, (r0, r1))
                  for (r0, r1) in U_CHUNKS[_p]]))

_PROG = None
TRACE = False
LAST_EXEC_NS = None
LAST_RESULTS = None


def _bn1_stats(v, w1):
    """Exact batch stats of einsum('bijc,oc->bijo', (v_i-v_j)^2, w1)."""
    Bv, Nv, _ = v.shape
    S1 = v.sum(1)
    S2 = (v ** 2).sum(1)
    P = np.einsum('bic,bid->bcd', v, v)
    Q = np.einsum('bic,bid->bcd', v ** 2, v)
    R = np.einsum('bic,bid->bcd', v ** 2, v ** 2)
    sim_sum = 2 * Nv * S2 - 2 * S1 ** 2
    M = (2 * Nv * R
         + 2 * np.einsum('bc,bd->bcd', S2, S2)
         + 4 * P ** 2
         - 4 * np.einsum('bcd,bd->bcd', Q, S1)
         - 4 * np.einsum('bdc,bc->bcd', Q, S1))
    n = Bv * Nv * Nv
    m1 = w1 @ (sim_sum.sum(0) / n)
    E2 = np.einsum('oc,cd,od->o', w1, M.sum(0) / n, w1)
    return m1, E2 - m1 ** 2


def build_program(n_cores=N_CORES, no_collective=False):
    nc = bacc.Bacc(None, target_bir_lowering=False, debug=False)

    def inp(name, shape, dt=F32):
        return nc.dram_tensor(name, list(shape), dt, kind="ExternalInput")

    VKS = ("mid", "pt")
    vshift = {vk: inp(f"vshift_{vk}", (128, BL, N), F16) for vk in VKS}
    vdup = {vk: inp(f"vdup_{vk}", (128, BL, N), F16) for vk in VKS}
    w1T = inp("w1T", (128, CH1), F16)
    w2T = inp("w2T", (CH1, C), F16)
    w3pair = inp("w3pair", (CH1, 2), F16)
    ab1 = {vk: inp(f"ab1_{vk}", (CH1, 2)) for vk in VKS}
    g2b2 = inp("g2b2", (C, 2))
    b3p = inp("b3p", (128, 1))
    point_edge = inp("point_edge", (BL, N, N))
    dnT0 = inp("dnT0", (S, BL, N))
    p2d_wa = inp("p2d_wa", (S, G, S))
    p2d_wb = inp("p2d_wb", (S, G, S))
    p2d_bias = inp("p2d_bias", (S, G))
    maskdiag = inp("maskdiag", (N, N))
    eyeplus = inp("eyeplus", (N, N))
    ident = inp("ident", (128, 128))

    out_dn = [nc.dram_tensor(f"out{l}", [BL, N, S], F32, kind="ExternalOutput")
              for l in range(G)]

    e_full = {vk: nc.dram_tensor(f"efull_{vk}", [BL, N, N], F32) for vk in VKS}
    cc_in = {vk: nc.dram_tensor(f"ccin_{vk}", [128, 2], F32) for vk in VKS}
    cc_out = {vk: nc.dram_tensor(f"ccout_{vk}", [128, 2], F32,
                                 addr_space="Shared") for vk in VKS}
    groups = [list(range(n_cores))]

    with tile.TileContext(nc) as tc, \
         tc.tile_pool(name="singles", bufs=1) as singles, \
         tc.tile_pool(name="hpt", bufs=1) as hptpool:

        dma = nc.default_dma_engine

        def load(t, shape, dt=F32, tag=None):
            sb = singles.tile(list(shape), dt, tag=tag or t.name,
                              name=tag or t.name)
            dma.dma_start(out=sb, in_=t[tuple(slice(0, s) for s in shape)])
            return sb

        vshift_sb = {vk: load(vshift[vk], (128, BL, N), F16) for vk in VKS}
        vdup_sb = {vk: load(vdup[vk], (128, BL, N), F16) for vk in VKS}
        w1T_sb = load(w1T, (128, CH1), F16)
        w2T_sb = load(w2T, (CH1, C), F16)
        w3p_sb = load(w3pair, (CH1, 2), F16)
        ab1_sb = {vk: load(ab1[vk], (CH1, 2)) for vk in VKS}
        g2b2_sb = load(g2b2, (C, 2))
        b3_sb = load(b3p, (128, 1))
        dnT0_sb = load(dnT0, (S, BL, N))
        p2dwa_sb = load(p2d_wa, (S, G, S))
        p2dwb_sb = load(p2d_wb, (S, G, S))
        p2db_sb = load(p2d_bias, (S, G))
        ident_sb = load(ident, (128, 128))
        mask_sb = [load(maskdiag, (128, N), tag="mask0"),
                   singles.tile([32, N], F32, tag="mask1", name="mask1")]
        dma.dma_start(out=mask_sb[1], in_=maskdiag[128:160, :])
        eyep_sb = [load(eyeplus, (128, N), tag="eyep0"),
                   singles.tile([32, N], F32, tag="eyep1", name="eyep1")]
        dma.dma_start(out=eyep_sb[1], in_=eyeplus[128:160, :])

        h_all = {vk: hptpool.tile([128, BL, FLAT], F16, tag=f"h_{vk}",
                                  name=f"h_{vk}")
                 for vk in VKS}
        stats_T = {vk: singles.tile([128, N_TCH * BL, 6], F32,
                                    tag=f"statsT_{vk}",
                                    name=f"statsT_{vk}") for vk in VKS}
        stats_U = {vk: singles.tile([128, N_UCH * BL, 6], F32,
                                    tag=f"statsU_{vk}",
                                    name=f"statsU_{vk}") for vk in VKS}

        with tc.tile_pool(name="wpa", bufs=3) as wpa, \
             tc.tile_pool(name="wpb", bufs=3) as wpb, \
             tc.tile_pool(name="pcp", bufs=2) as pcp, \
             tc.tile_pool(name="pck", bufs=1) as pck, \
             tc.tile_pool(name="pph1", bufs=2, space="PSUM") as pph1, \
             tc.tile_pool(name="pph2", bufs=2, space="PSUM") as pph2:

            # ---------------- pass A item ----------------
            def emit_a(vk, bl, widx):
                kind, p, poff, psz, chunks = WORK[widx]
                simtmp = wpa.tile([128, 1280], F16, tag="simtmp", bufs=6)
                sim = wpa.tile([128, 1280], F16, tag="sim", bufs=6)
                if kind == "T":
                    in0 = (vshift_sb[vk][:, bl, :]
                           .rearrange("c (p i) -> c p i", i=16)
                           [:, :, 0:8].unsqueeze(-1)
                           .broadcast_to([128, NBLK, 8, 16]))
                    in1 = (vdup_sb[vk][:, bl, :]
                           .rearrange("c (p w) -> c p w", w=16)
                           .unsqueeze(2)
                           .broadcast_to([128, NBLK, 8, 16]))
                    st = simtmp[:, :1280].rearrange(
                        "c (p i w) -> c p i w", i=8, w=16)
                    sv = sim[:, :1280].rearrange(
                        "c (p i w) -> c p i w", i=8, w=16)
                else:
                    w = WU[p]
                    in0 = (vshift_sb[vk][:, bl, 16 * p:16 * p + 8]
                           .unsqueeze(-1).broadcast_to([128, 8, w]))
                    in1 = (vdup_sb[vk][:, bl, 16 * p + 16:N]
                           .unsqueeze(1).broadcast_to([128, 8, w]))
                    st = simtmp[:, :8 * w].rearrange("c (i w) -> c i w", w=w)
                    sv = sim[:, :8 * w].rearrange("c (i w) -> c i w", w=w)
                sub_eng = nc.gpsimd if bl == 0 else nc.vector
                sub_eng.tensor_sub(st, in0, in1)
                nc.vector.tensor_mul(sv, st, st)

                hAB = wpa.tile([128, 2, 1280], F16, tag="hAB")
                hA = hAB[:, 0, :psz]
                hB = hAB[:, 1, :psz]
                for half, hdst in ((0, hA), (1, hB)):
                    rows = sim[64 * half:64 * half + 64, :psz]
                    for (c0, cw, _x) in chunks:
                        h1 = pph1.tile([128, 512], F32, tag="h1")
                        nc.tensor.matmul(
                            h1[:, :cw],
                            lhsT=w1T_sb[64 * half:64 * half + 64, :],
                            rhs=rows[:, c0:c0 + cw],
                            start=True, stop=True)
                        nc.scalar.activation(
                            out=hdst[:, c0:c0 + cw], in_=h1[:, :cw],
                            func=AF.Prelu,
                            bias=ab1_sb[vk][:, 1:2],
                            scale=ab1_sb[vk][:, 0:1],
                            alpha=SLOPE)

                h2d = h_all[vk][:, bl, poff:poff + psz]
                for k, (c0, cw, _x) in enumerate(chunks):
                    h2 = pph2.tile([128, 512], F32, tag="h2")
                    nc.tensor.matmul(h2[0:64, :cw], lhsT=w2T_sb,
                                     rhs=hA[:, c0:c0 + cw],
                                     start=True, stop=True)
                    nc.tensor.matmul(h2[64:128, :cw], lhsT=w2T_sb,
                                     rhs=hB[:, c0:c0 + cw],
                                     start=True, stop=True)
                    if k % 2 == 0:
                        nc.scalar.copy(h2d[:, c0:c0 + cw], h2[:, :cw])
                    else:
                        nc.vector.tensor_copy(h2d[:, c0:c0 + cw],
                                              h2[:, :cw])
                    if kind == "T":
                        dst = stats_T[vk][:, N_TCH * bl + k, :]
                    else:
                        dst = stats_U[vk][:, N_UCH * bl + U_SLOT[p] + k, :]
                    nc.vector.bn_stats(out=dst, in_=h2d[:, c0:c0 + cw])

            # ------------- stats reduce / collective -------------
            def reduce_sums(vk):
                nT = float(NT_POS * BL)
                nU2 = float(2 * NU_POS * BL)
                with tc.tile_pool(name=f"st_{vk}", bufs=1) as sp:
                    mvT = sp.tile([128, 2], F32, tag="mvT")
                    nc.vector.bn_aggr(out=mvT, in_=stats_T[vk])
                    mvU = sp.tile([128, 2], F32, tag="mvU")
                    nc.vector.bn_aggr(out=mvU, in_=stats_U[vk])
                    sums = sp.tile([128, 2], F32, tag="sums")
                    tmpU = sp.tile([128, 1], F32, tag="tmpU")
                    nc.vector.tensor_scalar_mul(tmpU, mvU[:, 0:1], nU2)
                    nc.vector.tensor_scalar_mul(sums[:, 0:1], mvT[:, 0:1], nT)
                    nc.vector.tensor_add(sums[:, 0:1], sums[:, 0:1], tmpU)
                    msqT = sp.tile([128, 1], F32, tag="msqT")
                    nc.vector.tensor_mul(msqT, mvT[:, 0:1], mvT[:, 0:1])
                    nc.vector.tensor_add(msqT, msqT, mvT[:, 1:2])
                    msqU = sp.tile([128, 1], F32, tag="msqU")
                    nc.vector.tensor_mul(msqU, mvU[:, 0:1], mvU[:, 0:1])
                    nc.vector.tensor_add(msqU, msqU, mvU[:, 1:2])
                    nc.vector.tensor_scalar_mul(msqT, msqT, nT)
                    nc.vector.tensor_scalar_mul(msqU, msqU, nU2)
                    nc.vector.tensor_add(sums[:, 1:2], msqT, msqU)
                    dma.dma_start(out=cc_in[vk][:, :], in_=sums)

            def collective(vk):
                if no_collective:
                    dma.dma_start(out=cc_out[vk][:, :], in_=cc_in[vk][:, :])
                else:
                    nc.gpsimd.collective_compute(
                        "AllReduce", ALU.add, replica_groups=groups,
                        ins=[cc_in[vk][:, :]], outs=[cc_out[vk][:, :]])

            # ------------- alpha2 / beta2 (gpsimd only: no queue stalls) ----
            def compute_ab2(vk):
                I32 = mybir.dt.int32
                with tc.tile_pool(name=f"ab2w_{vk}", bufs=1) as sp:
                    gs = sp.tile([128, 2], F32, tag="gs")
                    dma.dma_start(out=gs, in_=cc_out[vk][:, :])
                    bot = sp.tile([C, 2], F32, tag="bot")
                    dma.dma_start(out=bot, in_=gs[64:128, :])
                    tot = sp.tile([C, 2], F32, tag="tot")
                    nc.gpsimd.tensor_add(tot, gs[0:64, :], bot)
                    mE = sp.tile([C, 2], F32, tag="mE")
                    nc.gpsimd.tensor_scalar_mul(mE, tot, 1.0 / NTOT)
                    xe = sp.tile([C, 1], F32, tag="xe")
                    nc.gpsimd.tensor_mul(xe, mE[:, 0:1], mE[:, 0:1])
                    nc.gpsimd.tensor_sub(xe, mE[:, 1:2], xe)
                    nc.gpsimd.tensor_scalar_add(xe, xe, BN_EPS)
                    # rsqrt = exp(-0.5*ln(x)) + two Newton polishes on gpsimd
                    y = sp.tile([C, 1], F32, tag="y")
                    lx = sp.tile([C, 1], F32, tag="lx")
                    eps_t = sp.tile([C, 1], F32, tag="eps_t")
                    nc.gpsimd.memset(eps_t, 0.0)
                    nc.scalar.activation(out=lx, in_=xe, func=AF.Ln,
                                         bias=eps_t)
                    nc.gpsimd.tensor_scalar_mul(lx, lx, -0.5)
                    nc.scalar.activation(out=y, in_=lx, func=AF.Exp,
                                         bias=eps_t)
                    t1 = sp.tile([C, 1], F32, tag="t1")
                    for _ in range(2):
                        nc.gpsimd.tensor_mul(t1, y, y)
                        nc.gpsimd.tensor_mul(t1, t1, xe)
                        nc.gpsimd.tensor_scalar(t1, t1, -0.5, 1.5,
                                                ALU.mult, ALU.add)
                        nc.gpsimd.tensor_mul(y, y, t1)
                    ab2 = sp.tile([C, 2], F32, tag="ab2")
                    nc.gpsimd.tensor_mul(ab2[:, 0:1], y, g2b2_sb[:, 0:1])
                    t2 = sp.tile([C, 1], F32, tag="t2")
                    nc.gpsimd.tensor_mul(t2, mE[:, 0:1], ab2[:, 0:1])
                    nc.gpsimd.tensor_sub(ab2[:, 1:2], g2b2_sb[:, 1:2], t2)
                    ab2p = singles.tile([128, 2], F32, tag=f"ab2p_{vk}",
                                        name=f"ab2p_{vk}")
                    dma.dma_start(out=ab2p[0:64, :], in_=ab2)
                    dma.dma_start(out=ab2p[64:128, :], in_=ab2)
                return ab2p

            # ---------------- pass B item ----------------
            def emit_b(vk, ab2p, bl, widx):
                kind, p, poff, psz, chunks = WORK[widx]
                h2s = h_all[vk][:, bl, poff:poff + psz]
                for ci, (c0, cw, extra) in enumerate(chunks):
                    act_on_vec = ci % 2 == 1
                    hh = wpb.tile([128, 512], F16, tag="hh")
                    if not act_on_vec:
                        nc.scalar.activation(
                            out=hh[:, :cw], in_=h2s[:, c0:c0 + cw],
                            func=AF.Prelu,
                            bias=ab2p[:, 1:2], scale=ab2p[:, 0:1],
                            alpha=SLOPE)
                    else:
                        hy = wpb.tile([128, 512], F32, tag="hy")
                        nc.vector.tensor_scalar(hy[:, :cw],
                                                h2s[:, c0:c0 + cw],
                                                ab2p[:, 0:1], ab2p[:, 1:2],
                                                ALU.mult, ALU.add)
                        nc.vector.scalar_tensor_tensor(
                            out=hh[:, :cw], in0=hy[:, :cw], scalar=SLOPE,
                            in1=hy[:, :cw], op0=ALU.mult, op1=ALU.max)
                    e_pre = pph2.tile([2, 512], F32, tag="epre")
                    nc.tensor.matmul(e_pre[:, :cw], lhsT=w3p_sb,
                                     rhs=hh[:, :cw], start=True, stop=True)
                    e_sb = wpb.tile([2, 512], F32, tag="esb")
                    if act_on_vec:
                        nc.scalar.copy(e_sb[:, :cw], e_pre[:, :cw])
                    else:
                        nc.vector.tensor_copy(e_sb[:, :cw], e_pre[:, :cw])
                    if kind == "T":
                        for q in range(extra):
                            blk = c0 // 128 + q
                            dst = (e_full[vk]
                                   [bl, 16 * blk:16 * blk + 16,
                                    16 * blk:16 * blk + 16]
                                   .rearrange("(h i) w -> h i w", h=2))
                            src = (e_sb[:, 128 * q:128 * q + 128]
                                   .rearrange("h (i w) -> h i w", w=16))
                            dma.dma_start(out=dst, in_=src)
                    else:
                        w = WU[p]
                        r0, r1 = extra
                        dst = (e_full[vk]
                               [bl, 16 * p:16 * p + 16, 16 * p + 16:N]
                               .rearrange("(h i) w -> h i w", h=2)
                               [:, r0:r1, :])
                        src = e_sb[:, :cw].rearrange("h (i w) -> h i w", w=w)
                        dma.dma_start(out=dst, in_=src)

            # ------------- mirror item (lower triangle of e) -------------
            def emit_m(vk, bl, p):
                w = WU[p]
                strip = wpb.tile([16, 144], F32, tag="strip")
                dma.dma_start(out=strip[:, :w],
                              in_=e_full[vk][bl, 16 * p:16 * p + 16,
                                             16 * p + 16:N])
                woff = 0
                while woff < w:
                    wc = min(128, w - woff)
                    ps = pph2.tile([128, 512], F32, tag="h2r")
                    nc.tensor.matmul(ps[:wc, :16],
                                     lhsT=strip[:, woff:woff + wc],
                                     rhs=ident_sb[0:16, 0:16],
                                     is_transpose=True, start=True, stop=True)
                    so = wpb.tile([128, 16], F32, tag="strip_out")
                    nc.scalar.copy(so[:wc, :], ps[:wc, :16])
                    dma.dma_start(
                        out=e_full[vk][bl, 16 * p + 16 + woff:
                                       16 * p + 16 + woff + wc,
                                       16 * p:16 * p + 16],
                        in_=so[:wc, :])
                    woff += wc

            # ---------------- phase C helpers ----------------
            def epilogue(e_tiles, ep_tiles, tag):
                pe_tiles = []
                for blk, pdim in ((0, 128), (1, 32)):
                    e_t, ep_t = e_tiles[blk], ep_tiles[blk]
                    epm = pcp.tile([pdim, N], F32, tag=f"epm{blk}")
                    rs = pcp.tile([pdim, 1], F32, tag=f"rs{blk}")
                    nc.vector.scalar_tensor_tensor(
                        out=epm, in0=ep_t, scalar=1.0,
                        in1=mask_sb[blk][:pdim, :],
                        op0=ALU.mult, op1=ALU.mult, accum_out=rs)
                    x = pcp.tile([pdim, N], F32, tag=f"x{blk}_{tag}")
                    xs = pcp.tile([pdim, 1], F32, tag=f"xs{blk}")
                    nc.vector.scalar_tensor_tensor(
                        out=x, in0=e_t, scalar=1.0, in1=epm,
                        op0=ALU.mult, op1=ALU.mult, accum_out=xs)
                    nc.vector.tensor_scalar_max(xs, xs, 1e-12)
                    rxs = pcp.tile([pdim, 1], F32, tag=f"rxs{blk}")
                    nc.vector.reciprocal(rxs, xs)
                    nc.vector.tensor_mul(rxs, rxs, rs)
                    x2 = pcp.tile([pdim, N], F32, tag=f"x2{blk}_{tag}")
                    rs2 = pcp.tile([pdim, 1], F32, tag=f"rs2{blk}")
                    nc.vector.scalar_tensor_tensor(
                        out=x2, in0=x, scalar=rxs,
                        in1=eyep_sb[blk][:pdim, :],
                        op0=ALU.mult, op1=ALU.add, accum_out=rs2)
                    rrs2 = pcp.tile([pdim, 1], F32, tag=f"rrs2{blk}")
                    nc.vector.reciprocal(rrs2, rs2)
                    nc.vector.tensor_scalar_mul(x2, x2, rrs2)
                    pe_tiles.append(x2)
                return pe_tiles

            def pe_transpose(src_ap, pdim, fdim):
                ps = pph2.tile([128, 512], F32, tag="h2r")
                nc.tensor.matmul(ps[:fdim, :pdim], lhsT=src_ap,
                                 rhs=ident_sb[:pdim, :pdim],
                                 is_transpose=True, start=True, stop=True)
                dst = pcp.tile([fdim, pdim], F32, tag=f"tps{fdim}_{pdim}")
                nc.vector.tensor_copy(dst, ps[:fdim, :pdim])
                return dst

            CST = {}

            def load_blocks(bl, src, tagp, sigmoid=False):
                t0 = pck.tile([128, N], F32, tag=f"{tagp}0_{bl}")
                dma.dma_start(out=t0, in_=src[0:128, :])
                t1 = pck.tile([32, N], F32, tag=f"{tagp}1_{bl}")
                dma.dma_start(out=t1, in_=src[128:160, :])
                if sigmoid:
                    nc.scalar.activation(out=t0, in_=t0, func=AF.Sigmoid,
                                         bias=b3_sb[0:128])
                    nc.scalar.activation(out=t1, in_=t1, func=AF.Sigmoid,
                                         bias=b3_sb[0:32])
                return [t0, t1]

            def c_load_mid(bl):
                CST[(bl, "emid")] = load_blocks(
                    bl, e_full["mid"][bl], "emid", sigmoid=True)

            def c_load_pt(bl):
                CST[(bl, "ept")] = load_blocks(
                    bl, e_full["pt"][bl], "ept", sigmoid=True)

            def c_pe1(bl):
                CST[(bl, "pe1")] = epilogue(CST[(bl, "emid")],
                                            CST[(bl, "ep0")], f"pe1_{bl}")

            def c_pe2(bl):
                CST[(bl, "pe2")] = epilogue(CST[(bl, "ept")],
                                            CST[(bl, "pe1")], f"pe2_{bl}")

            def c_pe3(bl):
                CST[(bl, "pe3")] = epilogue(CST[(bl, "ept")],
                                            CST[(bl, "pe2")], f"pe3_{bl}")

            def c_p2d(bl, l):
                pe_t = CST[(bl, "pe2" if l == 0 else "pe3")]
                xT = pck.tile([S, N], F32, tag=f"xT_{bl}_{l}")
                t0 = pe_transpose(pe_t[0][:, 0:S], 128, S)
                nc.vector.tensor_copy(xT[:, 0:128], t0)
                t1 = pe_transpose(pe_t[1][:, 0:S], 32, S)
                nc.vector.tensor_copy(xT[:, 128:160], t1)
                dnT = dnT0_sb[:, bl, :] if l == 0 else CST[(bl, "dn")]
                mm = pph2.tile([128, 512], F32, tag="h2r")
                nc.tensor.matmul(mm[:S, :N], lhsT=p2dwa_sb[:, l, :], rhs=xT,
                                 start=True, stop=False)
                nc.tensor.matmul(mm[:S, :N], lhsT=p2dwb_sb[:, l, :], rhs=dnT,
                                 start=False, stop=True)
                dn_new = pck.tile([S, N], F32, tag=f"dnT_{bl}_{l}")
                dn_y = pck.tile([S, N], F32, tag=f"dny_{bl}_{l}")
                nc.vector.tensor_scalar_add(dn_y, mm[:S, :N],
                                            p2db_sb[:, l:l + 1])
                nc.vector.scalar_tensor_tensor(
                    out=dn_new, in0=dn_y, scalar=SLOPE, in1=dn_y,
                    op0=ALU.mult, op1=ALU.max)
                CST[(bl, "dn")] = dn_new
                o0 = pe_transpose(dn_new[:, 0:128], S, 128)
                dma.dma_start(out=out_dn[l][bl, 0:128, :], in_=o0)
                o1 = pe_transpose(dn_new[:, 128:160], S, 32)
                dma.dma_start(out=out_dn[l][bl, 128:160, :], in_=o1)

            # ---------------- schedule ----------------
            NW = len(WORK)
            A_items = [(bl, w) for bl in range(BL) for w in range(NW)]

            # prefetch point_edge (pure input) right away
            for bl in range(BL):
                CST[(bl, "ep0")] = load_blocks(bl, point_edge[bl], "ep0")

            for (bl, w) in A_items:
                emit_a("mid", bl, w)
            reduce_sums("mid")
            collective("mid")
            for (bl, w) in A_items:
                emit_a("pt", bl, w)
            reduce_sums("pt")
            collective("pt")
            ab2p_mid = compute_ab2("mid")

            # pass B mid, mirrors staggered 3 items behind their U writes
            for bl in range(BL):
                for w in range(NW):
                    emit_b("mid", ab2p_mid, bl, w)
                    if w >= 4:
                        emit_m("mid", bl, w - 4)
                for p in range(6, 9):
                    emit_m("mid", bl, p)
            c_load_mid(0)
            c_load_mid(1)

            ab2p_pt = compute_ab2("pt")
            for w in range(NW):
                emit_b("pt", ab2p_pt, 0, w)
                if w >= 4:
                    emit_m("pt", 0, w - 4)
            for p in range(6, 9):
                emit_m("pt", 0, p)
            c_pe1(0)
            c_pe1(1)
            c_load_pt(0)

            stages0 = [lambda: c_pe2(0), lambda: c_pe3(0),
                       lambda: c_p2d(0, 0), lambda: c_p2d(0, 1)]
            si = 0
            for j, w in enumerate(range(NW)):
                emit_b("pt", ab2p_pt, 1, w)
                if w >= 4:
                    emit_m("pt", 1, w - 4)
                if j % 2 == 1 and si < len(stages0):
                    stages0[si]()
                    si += 1
            for p in range(6, 9):
                emit_m("pt", 1, p)
            while si < len(stages0):
                stages0[si]()
                si += 1
            c_load_pt(1)
            c_pe2(1)
            c_pe3(1)
            c_p2d(1, 0)
            c_p2d(1, 1)

    nc.compile()
    return nc


def _prep_maps(middle_node, point_node, distribution_node, distribution_edge,
               point_edge, w1, g1, b1, w2, g2, b2, w3, b3, p2d_w, p2d_b,
               n_cores=N_CORES):
    f4 = np.float32
    middle_node = np.asarray(middle_node)
    point_node = np.asarray(point_node)

    def vt_pair(v_local):
        f2 = np.float16
        vT = np.transpose(v_local, (0, 2, 1)).astype(f2)      # [BL, C, N]
        sh = np.concatenate([vT[:, :, 8:], np.zeros((BL, C, 8), f2)], axis=2)
        vshift = np.concatenate([vT, sh], axis=1)             # [BL, 128, N]
        vdup = np.concatenate([vT, vT], axis=1)
        # -> [128, BL, N]
        return (np.ascontiguousarray(np.transpose(vshift, (1, 0, 2))),
                np.ascontiguousarray(np.transpose(vdup, (1, 0, 2))))

    def ab1_for(v):
        m1, var1 = _bn1_stats(v.astype(np.float64), np.asarray(w1, np.float64))
        a = np.asarray(g1, np.float64) / np.sqrt(var1 + BN_EPS)
        bb = np.asarray(b1, np.float64) - m1 * a
        return np.ascontiguousarray(np.stack([a, bb], axis=1).astype(f4))

    ab1_mid = ab1_for(middle_node)
    ab1_pt = ab1_for(point_node)

    w1T_h = np.ascontiguousarray(np.concatenate(
        [np.asarray(w1).T, np.asarray(w1).T], axis=0).astype(np.float16))
    w2T_h = np.ascontiguousarray(np.asarray(w2).T.astype(np.float16))
    w3pair_h = np.zeros((CH1, 2), np.float16)
    w3pair_h[0:C, 0] = np.asarray(w3).astype(np.float16)
    w3pair_h[C:CH1, 1] = np.asarray(w3).astype(np.float16)
    g2b2_h = np.ascontiguousarray(np.stack([np.asarray(g2), np.asarray(b2)],
                                           axis=1).astype(f4))
    b3p_h = np.full((128, 1), float(np.asarray(b3)), f4)
    pw = np.asarray(p2d_w)
    p2d_wa_h = np.ascontiguousarray(
        np.transpose(pw[:, :, 0:S], (2, 0, 1)).astype(f4))      # [S,G,S]
    p2d_wb_h = np.ascontiguousarray(
        np.transpose(pw[:, :, S:2 * S], (2, 0, 1)).astype(f4))  # [S,G,S]
    p2d_bias_h = np.ascontiguousarray(np.asarray(p2d_b).T.astype(f4))
    maskdiag_h = (1.0 - np.eye(N)).astype(f4)
    eyeplus_h = (np.eye(N) + 1e-6).astype(f4)
    ident_h = np.eye(128, dtype=f4)

    maps = []
    for c in range(n_cores):
        sl = slice(c * BL, (c + 1) * BL)
        vs_m, vd_m = vt_pair(middle_node[sl])
        vs_p, vd_p = vt_pair(point_node[sl])
        dnT0_h = np.ascontiguousarray(
            np.transpose(np.asarray(distribution_node)[sl], (2, 0, 1))
            .astype(f4))                                        # [S,BL,N]
        maps.append(dict(
            vshift_mid=vs_m, vdup_mid=vd_m, vshift_pt=vs_p, vdup_pt=vd_p,
            w1T=w1T_h, w2T=w2T_h, w3pair=w3pair_h,
            ab1_mid=ab1_mid, ab1_pt=ab1_pt, g2b2=g2b2_h, b3p=b3p_h,
            point_edge=np.ascontiguousarray(
                np.asarray(point_edge)[sl].astype(f4)),
            dnT0=dnT0_h, p2d_wa=p2d_wa_h, p2d_wb=p2d_wb_h,
            p2d_bias=p2d_bias_h,
            maskdiag=maskdiag_h, eyeplus=eyeplus_h, ident=ident_h,
        ))
    return maps


def kernel(**inputs):
    global _PROG, LAST_EXEC_NS, LAST_RESULTS
    if _PROG is None:
        _PROG = build_program()
    maps = _prep_maps(**inputs)
    res = run_bass_kernel_spmd(_PROG, maps, core_ids=list(range(N_CORES)),
                               trace=TRACE)
    LAST_EXEC_NS = res.exec_time_ns
    LAST_RESULTS = res
    outs = []
    for l in range(G):
        outs.append(np.concatenate([res.results[c][f"out{l}"]
                                    for c in range(N_CORES)], axis=0))
    return tuple(outs)

